# revision 31
# baseline (speedup 1.0000x reference)
"""Trainium2 Bass kernel for nn_Aggregation (gnn_message_passing).

Math (derived from the reference's faithful raw reshape):
  input_:  [n, 16, 16, 1024]  -> x[p, c], p = h*16+w in [0,256), c in [0,1024)
  weight:  [n, 256, 9, 256]   -> w_flat[q][u*256+l], q = b//4, u = (b%4)*9+t2
  out[n, c_out, l] with c_out = 4*p + m:
      out[p, m, l] = sum_{t2} w_flat[p%64][(m*9+t2)*256+l] * patch[p][(m*9+t2)*256+l]
  where patch[p][(4t+g)*256+l] = x[p + 16*(ki-1) + (kj-1)][g*256+l]  (t = ki*3+kj,
  zero at image borders).  So with Wr[p] := w_flat[p%64], the product tensor is a
  plain elementwise multiply prod = patch (*) Wr over 9216 contiguous elements per
  partition, and the t2-reduction sums u-blocks [9m, 9m+9) of 256-wide slices.

Per-core plan (8 cores, 4 samples each; partitions = p, two 128-row tiles,
fp16 on chip):
  - C1 = x loaded via SWDGE cast-DMA f32->fp16 into the middle of a [128,9216]
    "patch" tile that will hold all 9 shifted taps u-major.
  - C0/C2 (w-shifts +-1) and the 6 h-shifted taps (+-16) are built by PE
    matmuls against 0/1 shift matrices (border masking = zero matrix columns),
    evacuated PSUM(f32) -> patch(fp16) by ScalarE, two taps per PSUM tile.
  - Wr loaded by one [64,9216] cast-DMA; duplicated to partitions 64..127 by
    an SBUF->SBUF DMA.
  - DVE: contiguous fp16 multiplies (2x_1p mode) + 4-level pairwise add tree
    over the 9 u-blocks per m-chunk (contiguous slices, 2x mode).
  - Store: fully-contiguous per-tile DMA with fp16->f32 cast in the SWDGE.
"""

import os
import sys

import numpy as np

if "/opt/trn_rl_repo" not in sys.path:
    sys.path.insert(0, "/opt/trn_rl_repo")

import concourse.bacc as bacc
import concourse.mybir as mybir
from concourse.bass_utils import run_bass_kernel_spmd
from concourse.tile import TileContext

N_CORES = 8
S = 4            # samples per core
P = 256          # spatial positions per sample
CX = 1024        # input channels
WROW = 9216      # 4*9*256 weight row (and patch row) length
L = 256

USE_F32 = os.environ.get("AGG_F32", "0") == "1"
# fp16 over bf16: same 2-byte DVE perf modes, 3 extra mantissa bits;
# all on-chip values are O(30) so fp16 range is ample.
DT = mybir.dt.float32 if USE_F32 else mybir.dt.float16
F32 = mybir.dt.float32

_CACHED = {}


def _build_shift_matrices(nc, tc, pool):
    """0/1 shift matrices in SBUF, dtype DT.  mat[r, i] = 1 iff source row r
    feeds output row i (out = lhsT.T @ rhs with lhsT = mat)."""
    eq = mybir.AluOpType.is_equal
    mats = {}
    ones = pool.tile([128, 128], DT, tag="ones")
    nc.any.memset(ones[:], 1.0)

    def diag(name, base):
        # mat[r, i] = 1 iff r - i + base == 0   (i.e. r == i - base)
        m = pool.tile([128, 128], DT, tag=name)
        nc.gpsimd.affine_select(
            out=m[:], in_=ones[:], pattern=[[-1, 128]], compare_op=eq,
            fill=0.0, base=base, channel_multiplier=1,
        )
        mats[name] = m
        return m

    # w-shifts (+-1) with w-border masking folded in.
    # C2[i] = C1[i+1]  -> r == i+1 -> base=-1 ; mask columns i%16 == 15
    # keep where 14 - (i%16) >= 0
    m = diag("Ap1", -1)
    nc.gpsimd.affine_select(
        out=m[:].rearrange("p (a b) -> p a b", a=8, b=16),
        in_=m[:].rearrange("p (a b) -> p a b", a=8, b=16),
        pattern=[[0, 8], [-1, 16]], compare_op=mybir.AluOpType.is_ge,
        fill=0.0, base=14, channel_multiplier=0,
    )
    # C0[i] = C1[i-1]  -> r == i-1 -> base=+1 ; mask columns i%16 == 0
    # keep where (i%16) - 1 >= 0
    m = diag("Am1", 1)
    nc.gpsimd.affine_select(
        out=m[:].rearrange("p (a b) -> p a b", a=8, b=16),
        in_=m[:].rearrange("p (a b) -> p a b", a=8, b=16),
        pattern=[[0, 8], [1, 16]], compare_op=mybir.AluOpType.is_ge,
        fill=0.0, base=-1, channel_multiplier=0,
    )
    # h-shifts: E[i] = C[i +- 16], plus cross-tile boundary matrices.
    diag("Ap16", -16)   # r == i+16   (valid cols i < 112)
    diag("Am16", 16)    # r == i-16   (valid cols i >= 16)
    diag("Bp16", 112)   # r == i-112  (cols i >= 112, source = next tile)
    diag("Bm16", -112)  # r == i+112  (cols i < 16,  source = prev tile)
    return mats


def _build_module():
    nc = bacc.Bacc("TRN2", target_bir_lowering=False, debug=False,
                   num_devices=N_CORES)
    inp = nc.dram_tensor("input_", [S, P, CX], F32, kind="ExternalInput")
    wgt = nc.dram_tensor("weight", [S, 64, WROW], F32, kind="ExternalInput")
    out = nc.dram_tensor("out", [S, 4 * P, L], F32, kind="ExternalOutput")

    nb = 1 if USE_F32 else 2  # buffer count knob (f32 needs 2x SBUF per tile)

    with TileContext(nc) as tc:
        with (
            tc.tile_pool(name="consts", bufs=1) as cpool,
            tc.tile_pool(name="w", bufs=nb) as wpool,
            tc.tile_pool(name="c", bufs=nb) as cjpool,
            tc.tile_pool(name="prod", bufs=nb) as ppool,
            tc.tile_pool(name="tree", bufs=nb) as spool,
            tc.tile_pool(name="o", bufs=2) as opool,
            tc.tile_pool(name="ps", bufs=2, space="PSUM") as pspool,
        ):
            mats = _build_shift_matrices(nc, tc, cpool)

            for s in range(S):
                # ---- weight: load rows 0..63, duplicate to 64..127 ----
                wr = wpool.tile([128, WROW], DT, tag="wr")
                nc.gpsimd.dma_start(out=wr[0:64, :], in_=wgt[s])
                nc.sync.dma_start(out=wr[64:128, :], in_=wr[0:64, :])

                # ---- patch[d] = all 9 taps, u-major [128, 9216] ------------
                # taps 3,4,5 region [3072:6144] = [C0 | C1 | C2]:
                #   C1 direct cast-DMA load; C0/C2 via PE w-shift + strided evac.
                patch = []
                for d in range(2):
                    t = cjpool.tile([128, WROW], DT, tag=f"c_{d}")
                    nc.gpsimd.dma_start(out=t[:, 4 * CX:5 * CX],
                                        in_=inp[s, 128 * d:128 * (d + 1), :])
                    patch.append(t)

                def crhs(d, kj, h):  # PE rhs: C_kj half h of tile d
                    off = (3 + kj) * CX + 512 * h
                    return patch[d][:, off:off + 512]

                for d in range(2):
                    ps = pspool.tile([128, 2 * CX], F32, tag="ps")
                    for jj, mat in ((0, mats["Am1"]), (1, mats["Ap1"])):
                        for h in range(2):
                            nc.tensor.matmul(
                                ps[:, CX * jj + 512 * h:CX * jj + 512 * (h + 1)],
                                mat[:], crhs(d, 1, h),
                                start=True, stop=True,
                            )
                    # strided evac: psum [c0|c2] -> patch slots 3 and 5
                    nc.scalar.copy(
                        out=patch[d][:, 3 * CX:6 * CX].rearrange(
                            "p (a b) -> p a b", a=3)[:, 0:3:2, :],
                        in_=ps[:].rearrange("p (a b) -> p a b", a=2),
                    )

                # ---- per output tile: shifted taps via PE + evac ----
                # tap pairs sharing one [128, 2048] PSUM tile.
                # Order (0,1),(2,8) first so the first half of the multiply
                # (taps 0..5) can start while (6,7) is still in flight.
                TAP_PAIRS = [(0, 1), (2, 8), (6, 7)]
                for d in range(2):
                    for ta, tb in TAP_PAIRS:
                        ps = pspool.tile([128, 2 * CX], F32, tag="ps")
                        for slot, t in ((0, ta), (1, tb)):
                            ki, kj = divmod(t, 3)
                            if ki == 0:   # E[i] = C[i-16]
                                amat, bmat, dn = mats["Am16"], mats["Bm16"], d - 1
                            else:         # E[i] = C[i+16]
                                amat, bmat, dn = mats["Ap16"], mats["Bp16"], d + 1
                            has_b = 0 <= dn < 2
                            for h in range(2):
                                nc.tensor.matmul(
                                    ps[:, CX * slot + 512 * h:CX * slot + 512 * (h + 1)],
                                    amat[:], crhs(d, kj, h),
                                    start=True, stop=not has_b,
                                )
                            if has_b:
                                for h in range(2):
                                    nc.tensor.matmul(
                                        ps[:, CX * slot + 512 * h:CX * slot + 512 * (h + 1)],
                                        bmat[:], crhs(dn, kj, h),
                                        start=False, stop=True,
                                    )
                        if tb == ta + 1:
                            nc.scalar.copy(out=patch[d][:, CX * ta:CX * (ta + 2)],
                                           in_=ps[:])
                        else:
                            nc.scalar.copy(
                                out=patch[d][:, CX * ta:].rearrange(
                                    "p (a b) -> p a b", b=CX)[:, 0:tb - ta + 1:tb - ta, :],
                                in_=ps[:].rearrange("p (a b) -> p a b", a=2),
                            )

                for d in range(2):
                    prod = ppool.tile([128, WROW], DT, tag="prod")
                    # elementwise multiply prod = patch (*) Wr, split in two so
                    # the first half overlaps the (6,7) tap production.
                    # (keep on DVE — a GPSIMD tensor_tensor locks the shared
                    #  SBUF port pair and stalls concurrent DVE 2-input ops)
                    nc.vector.tensor_mul(out=prod[:, 0:6 * CX],
                                         in0=patch[d][:, 0:6 * CX],
                                         in1=wr[:, 0:6 * CX])
                    nc.vector.tensor_mul(out=prod[:, 6 * CX:],
                                         in0=patch[d][:, 6 * CX:],
                                         in1=wr[:, 6 * CX:])

                    # tree-reduce the 9 u-blocks of each 2304-elem m-chunk
                    pv = prod[:].rearrange("p (m r) -> p m r", m=4)
                    s1 = spool.tile([128, 4096], DT, tag="s1")
                    s2 = spool.tile([128, 2048], DT, tag="s2")
                    s3 = spool.tile([128, 1024], DT, tag="s3")
                    ot = opool.tile([128, 1024], DT, tag="ot")
                    s1v = s1[:].rearrange("p (m r) -> p m r", m=4)
                    s2v = s2[:].rearrange("p (m r) -> p m r", m=4)
                    nc.vector.tensor_add(
                        out=s1v, in0=pv[:, :, 0:1024], in1=pv[:, :, 1024:2048])
                    nc.vector.tensor_add(
                        out=s2v, in0=s1v[:, :, 0:512], in1=s1v[:, :, 512:1024])
                    nc.vector.tensor_add(
                        out=s3[:].rearrange("p (m r) -> p m r", m=4),
                        in0=s2v[:, :, 0:256], in1=s2v[:, :, 256:512])
                    nc.vector.tensor_add(
                        out=ot[:].rearrange("p (m r) -> p m r", m=4),
                        in0=s3[:].rearrange("p (m r) -> p m r", m=4),
                        in1=pv[:, :, 2048:2304])
                    # cast fp16 -> f32 happens inside the SWDGE store DMA
                    nc.gpsimd.dma_start(
                        out=out[s, 512 * d:512 * (d + 1), :]
                            .rearrange("(p f) l -> p (f l)", p=128),
                        in_=ot[:],
                    )
    nc.compile()
    return nc


def _get_nc():
    if "nc" not in _CACHED:
        _CACHED["nc"] = _build_module()
    return _CACHED["nc"]


def _shard(input_, weight):
    input_ = np.ascontiguousarray(input_, dtype=np.float32)
    weight = np.ascontiguousarray(weight, dtype=np.float32)
    n = input_.shape[0]
    per = n // N_CORES
    in_maps = []
    for cid in range(N_CORES):
        sl = slice(cid * per, (cid + 1) * per)
        in_maps.append({
            "input_": np.ascontiguousarray(input_[sl].reshape(per, P, CX)),
            "weight": np.ascontiguousarray(weight[sl].reshape(per, 64, WROW)),
        })
    return in_maps


def run(input_, weight, trace=False):
    nc = _get_nc()
    in_maps = _shard(input_, weight)
    res = run_bass_kernel_spmd(nc, in_maps, core_ids=list(range(N_CORES)),
                               trace=trace)
    outs = [r["out"].reshape(S, CX, 16, 16) for r in res.results]
    return np.concatenate(outs, axis=0), res


def kernel(input_, weight):
    out, _ = run(input_, weight, trace=False)
    return out


# revision 34
# speedup vs baseline: 1.1565x; 1.1565x over previous
"""Trainium2 Bass kernel for nn_Aggregation (gnn_message_passing).

Math (derived from the reference's faithful raw reshape):
  input_:  [n, 16, 16, 1024]  -> x[p, c], p = h*16+w in [0,256), c in [0,1024)
  weight:  [n, 256, 9, 256]   -> w_flat[q][u*256+l], q = b//4, u = (b%4)*9+t2
  out[n, c_out, l] with c_out = 4*p + m:
      out[p, m, l] = sum_{t2} w_flat[p%64][(m*9+t2)*256+l] * patch[p][(m*9+t2)*256+l]
  where patch[p][(4t+g)*256+l] = x[p + 16*(ki-1) + (kj-1)][g*256+l]  (t = ki*3+kj,
  zero at image borders).  So with Wr[p] := w_flat[p%64], the product tensor is a
  plain elementwise multiply prod = patch (*) Wr over 9216 contiguous elements per
  partition, and the t2-reduction sums u-blocks [9m, 9m+9) of 256-wide slices.

Per-core plan (8 cores, 4 samples each; partitions = p, two 128-row tiles,
fp16 on chip):
  - C1 = x loaded via SWDGE cast-DMA f32->fp16 into the middle of a [128,9216]
    "patch" tile that will hold all 9 shifted taps u-major.
  - C0/C2 (w-shifts +-1) and the 6 h-shifted taps (+-16) are built by PE
    matmuls against 0/1 shift matrices (border masking = zero matrix columns),
    evacuated PSUM(f32) -> patch(fp16) by ScalarE, two taps per PSUM tile.
  - Wr loaded by one [64,9216] cast-DMA; duplicated to partitions 64..127 by
    an SBUF->SBUF DMA.
  - DVE: contiguous fp16 multiplies (2x_1p mode) + 4-level pairwise add tree
    over the 9 u-blocks per m-chunk (contiguous slices, 2x mode).
  - Store: fully-contiguous per-tile DMA with fp16->f32 cast in the SWDGE.
"""

import os
import sys

import numpy as np

if "/opt/trn_rl_repo" not in sys.path:
    sys.path.insert(0, "/opt/trn_rl_repo")

import concourse.bacc as bacc
import concourse.mybir as mybir
from concourse.bass_utils import run_bass_kernel_spmd
from concourse.tile import TileContext

N_CORES = 8
S = 4            # samples per core
P = 256          # spatial positions per sample
CX = 1024        # input channels
WROW = 9216      # 4*9*256 weight row (and patch row) length
L = 256

USE_F32 = os.environ.get("AGG_F32", "0") == "1"
# fp16 over bf16: same 2-byte DVE perf modes, 3 extra mantissa bits;
# all on-chip values are O(30) so fp16 range is ample.
DT = mybir.dt.float32 if USE_F32 else mybir.dt.float16
F32 = mybir.dt.float32

_CACHED = {}


def _build_shift_matrices(nc, tc, pool):
    """0/1 shift matrices in SBUF, dtype DT.  mat[r, i] = 1 iff source row r
    feeds output row i (out = lhsT.T @ rhs with lhsT = mat)."""
    eq = mybir.AluOpType.is_equal
    mats = {}
    ones = pool.tile([128, 128], DT, tag="ones")
    nc.any.memset(ones[:], 1.0)

    def diag(name, base):
        # mat[r, i] = 1 iff r - i + base == 0   (i.e. r == i - base)
        m = pool.tile([128, 128], DT, tag=name)
        nc.gpsimd.affine_select(
            out=m[:], in_=ones[:], pattern=[[-1, 128]], compare_op=eq,
            fill=0.0, base=base, channel_multiplier=1,
        )
        mats[name] = m
        return m

    # w-shifts (+-1) with w-border masking folded in.
    # C2[i] = C1[i+1]  -> r == i+1 -> base=-1 ; mask columns i%16 == 15
    # keep where 14 - (i%16) >= 0
    m = diag("Ap1", -1)
    nc.gpsimd.affine_select(
        out=m[:].rearrange("p (a b) -> p a b", a=8, b=16),
        in_=m[:].rearrange("p (a b) -> p a b", a=8, b=16),
        pattern=[[0, 8], [-1, 16]], compare_op=mybir.AluOpType.is_ge,
        fill=0.0, base=14, channel_multiplier=0,
    )
    # C0[i] = C1[i-1]  -> r == i-1 -> base=+1 ; mask columns i%16 == 0
    # keep where (i%16) - 1 >= 0
    m = diag("Am1", 1)
    nc.gpsimd.affine_select(
        out=m[:].rearrange("p (a b) -> p a b", a=8, b=16),
        in_=m[:].rearrange("p (a b) -> p a b", a=8, b=16),
        pattern=[[0, 8], [1, 16]], compare_op=mybir.AluOpType.is_ge,
        fill=0.0, base=-1, channel_multiplier=0,
    )
    # h-shifts: E[i] = C[i +- 16], plus cross-tile boundary matrices.
    diag("Ap16", -16)   # r == i+16   (valid cols i < 112)
    diag("Am16", 16)    # r == i-16   (valid cols i >= 16)
    diag("Bp16", 112)   # r == i-112  (cols i >= 112, source = next tile)
    diag("Bm16", -112)  # r == i+112  (cols i < 16,  source = prev tile)
    return mats


def _build_module():
    nc = bacc.Bacc("TRN2", target_bir_lowering=False, debug=False,
                   num_devices=N_CORES)
    inp = nc.dram_tensor("input_", [S, P, CX], F32, kind="ExternalInput")
    wgt = nc.dram_tensor("weight", [S, 64, WROW], F32, kind="ExternalInput")
    out = nc.dram_tensor("out", [S, 4 * P, L], F32, kind="ExternalOutput")

    nb = 1 if USE_F32 else 2  # buffer count knob (f32 needs 2x SBUF per tile)

    with TileContext(nc) as tc:
        with (
            tc.tile_pool(name="consts", bufs=1) as cpool,
            tc.tile_pool(name="w", bufs=nb) as wpool,
            tc.tile_pool(name="c", bufs=nb) as cjpool,
            tc.tile_pool(name="prod", bufs=nb) as ppool,
            tc.tile_pool(name="tree", bufs=nb) as spool,
            tc.tile_pool(name="o", bufs=2) as opool,
            tc.tile_pool(name="ps", bufs=2, space="PSUM") as pspool,
        ):
            mats = _build_shift_matrices(nc, tc, cpool)

            for s in range(S):
                # ---- weight: load rows 0..63, duplicate to 64..127 ----
                wr = wpool.tile([128, WROW], DT, tag="wr")
                nc.gpsimd.dma_start(out=wr[0:64, :], in_=wgt[s])
                nc.sync.dma_start(out=wr[64:128, :], in_=wr[0:64, :])

                # ---- patch[d] = all 9 taps, u-major [128, 9216] ------------
                # taps 3,4,5 region [3072:6144] = [C0 | C1 | C2]:
                #   C1 direct cast-DMA load; C0/C2 via PE w-shift + strided evac.
                # taps 1 (x[p-16]) and 7 (x[p+16]) have pure h-edge masking
                # (contiguous partition blocks) -> load straight from HBM as
                # shifted contiguous cast-DMAs + one [16, CX] edge memzero.
                patch = []
                for d in range(2):
                    t = cjpool.tile([128, WROW], DT, tag=f"c_{d}")
                    nc.gpsimd.dma_start(out=t[:, 4 * CX:5 * CX],
                                        in_=inp[s, 128 * d:128 * (d + 1), :])
                    if d == 0:
                        # zero aligned [0:32), DMA then overwrites [16:32)
                        nc.scalar.memzero(t[0:32, 1 * CX:2 * CX])
                        nc.gpsimd.dma_start(out=t[16:128, 1 * CX:2 * CX],
                                            in_=inp[s, 0:112, :])
                        nc.gpsimd.dma_start(out=t[:, 7 * CX:8 * CX],
                                            in_=inp[s, 16:144, :])
                    else:
                        nc.gpsimd.dma_start(out=t[:, 1 * CX:2 * CX],
                                            in_=inp[s, 112:240, :])
                        # zero aligned [96:128), DMA then overwrites [96:112)
                        nc.scalar.memzero(t[96:128, 7 * CX:8 * CX])
                        nc.gpsimd.dma_start(out=t[0:112, 7 * CX:8 * CX],
                                            in_=inp[s, 144:256, :])
                    patch.append(t)

                def crhs(d, kj, h):  # PE rhs: C_kj half h of tile d
                    off = (3 + kj) * CX + 512 * h
                    return patch[d][:, off:off + 512]

                for d in range(2):
                    ps = pspool.tile([128, 2 * CX], F32, tag="ps")
                    for jj, mat in ((0, mats["Am1"]), (1, mats["Ap1"])):
                        for h in range(2):
                            nc.tensor.matmul(
                                ps[:, CX * jj + 512 * h:CX * jj + 512 * (h + 1)],
                                mat[:], crhs(d, 1, h),
                                start=True, stop=True,
                            )
                    # strided evac: psum [c0|c2] -> patch slots 3 and 5
                    nc.scalar.copy(
                        out=patch[d][:, 3 * CX:6 * CX].rearrange(
                            "p (a b) -> p a b", a=3)[:, 0:3:2, :],
                        in_=ps[:].rearrange("p (a b) -> p a b", a=2),
                    )

                # ---- per output tile: remaining shifted taps via PE + evac --
                # taps 0,2,6,8 in pairs sharing one [128, 2048] PSUM tile,
                # written back to patch with a stride-6 two-block evac.
                TAP_PAIRS = [(0, 6), (2, 8)]
                for d in range(2):
                    for ta, tb in TAP_PAIRS:
                        ps = pspool.tile([128, 2 * CX], F32, tag="ps")
                        for slot, t in ((0, ta), (1, tb)):
                            ki, kj = divmod(t, 3)
                            if ki == 0:   # E[i] = C[i-16]
                                amat, bmat, dn = mats["Am16"], mats["Bm16"], d - 1
                            else:         # E[i] = C[i+16]
                                amat, bmat, dn = mats["Ap16"], mats["Bp16"], d + 1
                            has_b = 0 <= dn < 2
                            for h in range(2):
                                nc.tensor.matmul(
                                    ps[:, CX * slot + 512 * h:CX * slot + 512 * (h + 1)],
                                    amat[:], crhs(d, kj, h),
                                    start=True, stop=not has_b,
                                )
                            if has_b:
                                for h in range(2):
                                    nc.tensor.matmul(
                                        ps[:, CX * slot + 512 * h:CX * slot + 512 * (h + 1)],
                                        bmat[:], crhs(dn, kj, h),
                                        start=False, stop=True,
                                    )
                        if tb == ta + 1:
                            nc.scalar.copy(out=patch[d][:, CX * ta:CX * (ta + 2)],
                                           in_=ps[:])
                        else:
                            nc.scalar.copy(
                                out=patch[d][:, CX * ta:].rearrange(
                                    "p (a b) -> p a b", b=CX)[:, 0:tb - ta + 1:tb - ta, :],
                                in_=ps[:].rearrange("p (a b) -> p a b", a=2),
                            )

                for d in range(2):
                    prod = ppool.tile([128, WROW], DT, tag="prod")
                    # elementwise multiply prod = patch (*) Wr, split in two so
                    # the first half overlaps the (6,7) tap production.
                    # (keep on DVE — a GPSIMD tensor_tensor locks the shared
                    #  SBUF port pair and stalls concurrent DVE 2-input ops)
                    nc.vector.tensor_mul(out=prod[:, 0:6 * CX],
                                         in0=patch[d][:, 0:6 * CX],
                                         in1=wr[:, 0:6 * CX])
                    nc.vector.tensor_mul(out=prod[:, 6 * CX:],
                                         in0=patch[d][:, 6 * CX:],
                                         in1=wr[:, 6 * CX:])

                    # tree-reduce the 9 u-blocks of each 2304-elem m-chunk
                    pv = prod[:].rearrange("p (m r) -> p m r", m=4)
                    s1 = spool.tile([128, 4096], DT, tag="s1")
                    s2 = spool.tile([128, 2048], DT, tag="s2")
                    s3 = spool.tile([128, 1024], DT, tag="s3")
                    ot = opool.tile([128, 1024], DT, tag="ot")
                    s1v = s1[:].rearrange("p (m r) -> p m r", m=4)
                    s2v = s2[:].rearrange("p (m r) -> p m r", m=4)
                    nc.vector.tensor_add(
                        out=s1v, in0=pv[:, :, 0:1024], in1=pv[:, :, 1024:2048])
                    nc.vector.tensor_add(
                        out=s2v, in0=s1v[:, :, 0:512], in1=s1v[:, :, 512:1024])
                    nc.vector.tensor_add(
                        out=s3[:].rearrange("p (m r) -> p m r", m=4),
                        in0=s2v[:, :, 0:256], in1=s2v[:, :, 256:512])
                    nc.vector.tensor_add(
                        out=ot[:].rearrange("p (m r) -> p m r", m=4),
                        in0=s3[:].rearrange("p (m r) -> p m r", m=4),
                        in1=pv[:, :, 2048:2304])
                    # cast fp16 -> f32 happens inside the SWDGE store DMA
                    nc.gpsimd.dma_start(
                        out=out[s, 512 * d:512 * (d + 1), :]
                            .rearrange("(p f) l -> p (f l)", p=128),
                        in_=ot[:],
                    )
    nc.compile()
    return nc


def _get_nc():
    if "nc" not in _CACHED:
        _CACHED["nc"] = _build_module()
    return _CACHED["nc"]


def _shard(input_, weight):
    input_ = np.ascontiguousarray(input_, dtype=np.float32)
    weight = np.ascontiguousarray(weight, dtype=np.float32)
    n = input_.shape[0]
    per = n // N_CORES
    in_maps = []
    for cid in range(N_CORES):
        sl = slice(cid * per, (cid + 1) * per)
        in_maps.append({
            "input_": np.ascontiguousarray(input_[sl].reshape(per, P, CX)),
            "weight": np.ascontiguousarray(weight[sl].reshape(per, 64, WROW)),
        })
    return in_maps


def run(input_, weight, trace=False):
    nc = _get_nc()
    in_maps = _shard(input_, weight)
    res = run_bass_kernel_spmd(nc, in_maps, core_ids=list(range(N_CORES)),
                               trace=trace)
    outs = [r["out"].reshape(S, CX, 16, 16) for r in res.results]
    return np.concatenate(outs, axis=0), res


def kernel(input_, weight):
    out, _ = run(input_, weight, trace=False)
    return out
